# revision 34
# baseline (speedup 1.0000x reference)
"""Trainium2 Bass kernel for nn_AttnDecoderRNN (B=1024, S=100, H=256, E=128, V=50000).

Sharding across 8 NeuronCores:
  - batch-parallel (128 rows/core) for embedding gather, GRU cell, additive
    attention, out-hidden projection, p_gen;
  - vocab-parallel (6250 cols/core) for the vocab matmul + softmax:
    AllGather of the transposed decoder output, per-row-tile AllReduce of
    softmax denominators.

Self-contained: hardcodes all shapes; imports only concourse + numpy.
"""
import numpy as np
from contextlib import ExitStack

import concourse.bass as bass
import concourse.bacc as bacc
import concourse.tile as tile
import concourse.mybir as mybir
from concourse import bass_utils
from concourse.masks import make_identity

F32 = mybir.dt.float32
F32R = mybir.dt.float32r
BF16 = mybir.dt.bfloat16
FP16 = mybir.dt.float16
I32 = mybir.dt.int32
AX = mybir.AxisListType
ALU = mybir.AluOpType
ACTF = mybir.ActivationFunctionType

NC = 8           # cores
B = 1024         # batch
BL = B // NC     # batch rows per core (=128)
S = 100          # encoder length
H = 256          # hidden
E = 128          # embedding dim
V = 50000        # vocab
VL = V // NC     # vocab cols per core (=6250)
P = 128          # partitions
HT = H // P      # h tiles (=2)
G = 3 * H        # gru gate width (=768)

BC = 32          # attention batch chunk
NBC = BL // BC   # = 8
SCN = 4 * S      # scores matmul N-chunk (4 batch rows = 400)
VCH = 512        # vocab matmul N chunk
NVCH = (VL + VCH - 1) // VCH   # = 13 (12x512 + 106)

_CACHE = {}


def build_bass():
    nc = bacc.Bacc("TRN2", target_bir_lowering=False, debug=False, num_devices=NC)

    # ---------------- I/O ----------------
    tok = nc.dram_tensor("tok", [BL, 1], I32, kind="ExternalInput")
    emb_t = nc.dram_tensor("emb", [V, E], F32, kind="ExternalInput")
    h_t = nc.dram_tensor("h0", [BL, H], F32, kind="ExternalInput")
    encT_t = nc.dram_tensor("encT", [H, BL, S], FP16, kind="ExternalInput")
    wihT_t = nc.dram_tensor("wihT", [E, G], F32R, kind="ExternalInput")
    whhT_t = nc.dram_tensor("whhT", [HT, P, G], F32R, kind="ExternalInput")
    bih_t = nc.dram_tensor("bih", [1, G], F32, kind="ExternalInput")
    bhh_t = nc.dram_tensor("bhh", [1, G], F32, kind="ExternalInput")
    wh2_t = nc.dram_tensor("wh2", [P, HT], F32, kind="ExternalInput")
    ws2_t = nc.dram_tensor("ws2", [P, HT], F32, kind="ExternalInput")
    av2_t = nc.dram_tensor("av2", [P, HT], F32, kind="ExternalInput")
    attb_t = nc.dram_tensor("attb", [1, 1], F32, kind="ExternalInput")
    genw_t = nc.dram_tensor("genw", [P, 5], F32R, kind="ExternalInput")
    genb_t = nc.dram_tensor("genb", [1, 1], F32, kind="ExternalInput")
    outhWT_t = nc.dram_tensor("outhWT", [4, P, H], F32R, kind="ExternalInput")
    outhb_t = nc.dram_tensor("outhb", [1, H], F32, kind="ExternalInput")
    wvT_t = nc.dram_tensor("wvT", [H, VL], FP16, kind="ExternalInput")
    bv_t = nc.dram_tensor("bv", [1, VL], FP16, kind="ExternalInput")

    hn_o = nc.dram_tensor("hn_o", [BL, H], F32, kind="ExternalOutput")
    pg_o = nc.dram_tensor("pg_o", [BL, 1], F32, kind="ExternalOutput")
    ad_o = nc.dram_tensor("ad_o", [BL, S], F32, kind="ExternalOutput")
    pv_o = nc.dram_tensor("pv_o", [B, VL], F32, kind="ExternalOutput")

    with tile.TileContext(nc) as tc, ExitStack() as ctx:
        const = ctx.enter_context(tc.tile_pool(name="const", bufs=1))
        tp_ps = ctx.enter_context(tc.tile_pool(name="tp_ps", bufs=1, space="PSUM"))
        mm_ps = ctx.enter_context(tc.tile_pool(name="mm_ps", bufs=6, space="PSUM"))
        dram = ctx.enter_context(tc.tile_pool(name="dram", bufs=1, space="DRAM"))

        ident = const.tile([P, P], F32)
        make_identity(nc, ident[:])

        def transpose128(dst_ap, src_ap):
            """dst[j,i] = src[i,j] for a [128,128] block (via PE + DVE copy)."""
            ps = tp_ps.tile([P, P], F32, tag="tp", name="tp")
            nc.tensor.transpose(ps[:], src_ap, ident[:])
            nc.vector.tensor_copy(dst_ap, ps[:])

        # DRAM bounce buffers
        sc_dram = dram.tile([1, BL * S], F32, name="sc_dram")
        ad_dram = dram.tile([BL, S], FP16, name="ad_dram")
        ag_in = dram.tile([HT * 2 * P, P], FP16, name="ag_in")
        ag_out = dram.tile([NC * HT * 2 * P, P], FP16, name="ag_out")
        den_in = [dram.tile([P, 1], F32, name=f"den_in{r}") for r in range(NC)]
        den_out = [dram.tile([P, 1], F32, name=f"den_out{r}") for r in range(NC)]

        # tiles that must survive into the vocab phase
        decTall = const.tile([P, HT, 2, NC * P], FP16)
        ones_r = const.tile([1, P], F32R)

        with tc.tile_pool(name="w", bufs=1) as sb:
            # ---------------- prologue loads ----------------
            tok_sb = sb.tile([P, 1], I32)
            nc.sync.dma_start(tok_sb[:], tok.ap())
            x_sb = sb.tile([P, E], F32)
            nc.gpsimd.indirect_dma_start(
                out=x_sb[:], out_offset=None, in_=emb_t.ap(),
                in_offset=bass.IndirectOffsetOnAxis(ap=tok_sb[:, :1], axis=0))

            h_sb = sb.tile([P, H], F32)
            nc.sync.dma_start(h_sb[:], h_t.ap())
            wih_sb = sb.tile([P, G], F32R)
            nc.sync.dma_start(wih_sb[:], wihT_t.ap())
            whh_sb = sb.tile([P, HT, G], F32R)
            nc.sync.dma_start(whh_sb[:], whhT_t.ap().rearrange("t p n -> p t n"))
            wh2_sb = sb.tile([P, HT], F32)
            nc.sync.dma_start(wh2_sb[:], wh2_t.ap())
            ws2_sb = sb.tile([P, HT], F32)
            nc.sync.dma_start(ws2_sb[:], ws2_t.ap())
            av2_sb = sb.tile([P, HT], F32)
            nc.sync.dma_start(av2_sb[:], av2_t.ap())
            genw_sb = sb.tile([P, 5], F32R)
            nc.sync.dma_start(genw_sb[:], genw_t.ap())
            outhW_sb = sb.tile([P, 4, H], F32R)
            nc.sync.dma_start(outhW_sb[:], outhWT_t.ap().rearrange("t p n -> p t n"))

            NR = 2 * G + H + 2
            gru_ctx = ExitStack()
            grup = gru_ctx.enter_context(tc.tile_pool(name="grup", bufs=1))
            rows_sb = grup.tile([1, NR], F32)  # bih | bhh | outhb | attb | genb
            nc.sync.dma_start(rows_sb[:1, 0:G], bih_t.ap())
            nc.sync.dma_start(rows_sb[:1, G:2 * G], bhh_t.ap())
            nc.sync.dma_start(rows_sb[:1, 2 * G:2 * G + H], outhb_t.ap())
            nc.sync.dma_start(rows_sb[:1, 2 * G + H:2 * G + H + 1], attb_t.ap())
            nc.sync.dma_start(rows_sb[:1, 2 * G + H + 1:NR], genb_t.ap())

            ones_f = grup.tile([1, P], F32)
            nc.vector.memset(ones_f[:], 1.0)
            nc.vector.tensor_copy(ones_r[:], ones_f[:])

            # replicated bias tiles
            brz_row = grup.tile([1, 2 * H], F32)   # (b_ih + b_hh)[0:512]
            nc.vector.tensor_add(brz_row[:1, :], rows_sb[:1, 0:2 * H],
                                 rows_sb[:1, G:G + 2 * H])
            brz_rep = grup.tile([P, 2 * H], F32)
            nc.gpsimd.partition_broadcast(brz_rep[:], brz_row[:1, :])
            bihn_rep = grup.tile([P, H], F32)
            nc.gpsimd.partition_broadcast(bihn_rep[:], rows_sb[:1, 2 * H:G])
            bhhn_rep = grup.tile([P, H], F32)
            nc.gpsimd.partition_broadcast(bhhn_rep[:], rows_sb[:1, G + 2 * H:2 * G])
            outhb_rep = sb.tile([P, H], F32)
            nc.gpsimd.partition_broadcast(outhb_rep[:], rows_sb[:1, 2 * G:2 * G + H])
            attb_pp = sb.tile([P, 1], F32)
            nc.gpsimd.partition_broadcast(attb_pp[:],
                                          rows_sb[:1, 2 * G + H:2 * G + H + 1])
            genb_pp = sb.tile([P, 1], F32)
            nc.gpsimd.partition_broadcast(genb_pp[:], rows_sb[:1, 2 * G + H + 1:NR])

            # attn_v replicated into lhsT form [P(h), HT, 128(m)]
            av_rep = sb.tile([P, HT, P], FP16)
            for t in range(HT):
                nc.vector.tensor_copy(av_rep[:, t, :],
                                      av2_sb[:, t:t + 1].broadcast_to([P, P]))

            # ---------------- GRU ----------------
            xT_sb = sb.tile([P, E], F32R)
            transpose128(xT_sb[:], x_sb[:])
            hT_sb = sb.tile([P, HT, P], F32R)
            for t in range(HT):
                transpose128(hT_sb[:, t, :], h_sb[:, t * P:(t + 1) * P])

            hn_sb = grup.tile([P, H], F32)
            # gate chunks: a = cols [0:512] (r,z), b = cols [512:768] (n)
            gi_a = mm_ps.tile([P, VCH], F32, tag="mm", name="gi_a")
            gi_b = mm_ps.tile([P, VCH], F32, tag="mm", name="gi_b")
            gh_a = mm_ps.tile([P, VCH], F32, tag="mm", name="gh_a")
            gh_b = mm_ps.tile([P, VCH], F32, tag="mm", name="gh_b")
            for (pi, ph, n0, n1) in [(gi_a, gh_a, 0, 512), (gi_b, gh_b, 512, G)]:
                w = n1 - n0
                nc.tensor.matmul(pi[:, :w], xT_sb[:], wih_sb[:, n0:n1],
                                 start=True, stop=True)
                for t in range(HT):
                    nc.tensor.matmul(ph[:, :w], hT_sb[:, t, :],
                                     whh_sb[:, t, n0:n1],
                                     start=(t == 0), stop=(t == HT - 1))

            # r,z = sigmoid(gi + gh + bih + bhh) over [0:512]
            rz_sb = grup.tile([P, 2 * H], F32)
            nc.scalar.copy(rz_sb[:], gi_a[:, :2 * H])
            nc.vector.tensor_add(rz_sb[:], rz_sb[:], gh_a[:, :2 * H])
            nc.vector.tensor_add(rz_sb[:], rz_sb[:], brz_rep[:])
            # sigmoid(x) = 0.5*tanh(0.5*x) + 0.5 (keeps ACT on the Tanh table)
            nc.scalar.activation(rz_sb[:], rz_sb[:], ACTF.Tanh, scale=0.5)
            nc.vector.tensor_scalar(rz_sb[:], rz_sb[:], 0.5, 0.5,
                                    op0=ALU.mult, op1=ALU.add)
            # n = tanh(i_n + b_ihn + r*(h_n + b_hhn))
            n_sb = grup.tile([P, H], F32)
            nc.vector.tensor_add(n_sb[:], gh_b[:, :H], bhhn_rep[:])
            nc.vector.tensor_mul(n_sb[:], n_sb[:], rz_sb[:, 0:H])
            nc.vector.tensor_add(n_sb[:], n_sb[:], gi_b[:, :H])
            nc.vector.tensor_add(n_sb[:], n_sb[:], bihn_rep[:])
            nc.scalar.activation(n_sb[:], n_sb[:], ACTF.Tanh)
            # h_new = n + z*(h - n)
            nc.vector.tensor_sub(hn_sb[:], h_sb[:], n_sb[:])
            nc.vector.tensor_mul(hn_sb[:], hn_sb[:], rz_sb[:, H:2 * H])
            nc.vector.tensor_add(hn_sb[:], hn_sb[:], n_sb[:])
            nc.scalar.dma_start(hn_o.ap(), hn_sb[:])

            hnT_sb = sb.tile([P, HT, P], F32R)
            for t in range(HT):
                transpose128(hnT_sb[:, t, :], hn_sb[:, t * P:(t + 1) * P])
            # u'_T[h, b] = (w_s/w_h)*h_newT + att_bias/w_h so that
            # tanh(w_h*E + u) == tanh(w_h*(E + u')): w_h moves into ACT scale
            rwh2 = sb.tile([P, HT], F32)
            nc.vector.reciprocal(rwh2[:], wh2_sb[:])
            wsw = sb.tile([P, HT], F32)
            nc.vector.tensor_mul(wsw[:], ws2_sb[:], rwh2[:])
            abw = sb.tile([P, HT], F32)
            nc.vector.tensor_mul(abw[:], attb_pp[:, :1].broadcast_to([P, HT]),
                                 rwh2[:])
            uT_sb = sb.tile([P, HT, P], FP16)
            for t in range(HT):
                nc.vector.tensor_scalar(uT_sb[:, t, :], hnT_sb[:, t, :],
                                        wsw[:, t:t + 1], abw[:, t:t + 1],
                                        op0=ALU.mult, op1=ALU.add)

            gru_ctx.close()

            # ---------------- attention ----------------
            # processed in NBC independent batch chunks of BC rows:
            # scores -> per-chunk softmax -> context, streaming encoder chunks
            ctxT_f = sb.tile([P, HT, P], F32)
            ctxT_sb = sb.tile([P, HT, P], F32R)
            CSZ = BC * S  # 1600

            with tc.tile_pool(name="enc", bufs=4) as encp, \
                 tc.tile_pool(name="attw", bufs=2) as attp:
                esrc = encT_t.ap().rearrange("(t p) b s -> p t (b s)", t=HT)
                enc_tiles = []
                for c in range(NBC):
                    cs = slice(c * CSZ, (c + 1) * CSZ)
                    enc_c = encp.tile([P, HT, CSZ], FP16, tag="encc",
                                      name=f"enc_c{c}")
                    nc.sync.dma_start(enc_c[:], esrc[:, :, cs])
                    enc_tiles.append(enc_c)
                for c in range(NBC):
                    bsl = slice(c * BC, (c + 1) * BC)
                    enc_c = enc_tiles[c]
                    att_c = attp.tile([P, HT, CSZ], FP16, tag="attc", name="att_c")
                    for t in range(HT):
                        # att = tanh(w_h*(E + u'))  (u' broadcast over s;
                        # w_h applied via the per-partition ACT scale)
                        nc.vector.tensor_tensor(
                            att_c[:, t, :].rearrange("p (b s) -> p b s", b=BC),
                            enc_c[:, t, :].rearrange("p (b s) -> p b s", b=BC),
                            uT_sb[:, t, bsl].unsqueeze(2)
                                .broadcast_to([P, BC, S]),
                            op=ALU.add)
                        nc.scalar.activation(att_c[:, t, :], att_c[:, t, :],
                                             ACTF.Tanh,
                                             scale=wh2_sb[:, t:t + 1])
                    # scores chunk: av . att -> rows replicated; keep row 0
                    for k, n0 in enumerate(range(0, CSZ, SCN)):
                        sc_ps = mm_ps.tile([P, VCH], F32, tag="mm", name="sc_ps")
                        for t in range(HT):
                            nc.tensor.matmul(sc_ps[:, :SCN], av_rep[:, t, :],
                                             att_c[:, t, n0:n0 + SCN],
                                             start=(t == 0), stop=(t == HT - 1))
                        sc_row = attp.tile([1, SCN], F32, tag="scrow", name="sc_row")
                        if k % 2:
                            nc.vector.tensor_copy(sc_row[:1, :], sc_ps[:1, :SCN])
                        else:
                            nc.scalar.copy(sc_row[:1, :], sc_ps[:1, :SCN])
                        off = c * CSZ + n0
                        nc.scalar.dma_start(sc_dram[:1, off:off + SCN],
                                            sc_row[:1, :])

                    # scatter scores [1, CSZ] -> [BC, S] and per-chunk softmax
                    scores_c = attp.tile([BC, S], F32, tag="scc", name="scores_c")
                    nc.scalar.dma_start(
                        scores_c[:],
                        sc_dram[:1, c * CSZ:(c + 1) * CSZ]
                            .rearrange("o (b s) -> (o b) s", b=BC))
                    negmx = attp.tile([BC, 1], F32, tag="ngm", name="negmx")
                    nc.vector.tensor_reduce(negmx[:], scores_c[:], axis=AX.X,
                                            op=ALU.max, negate=True)
                    ssum = attp.tile([BC, 1], F32, tag="ssum", name="ssum")
                    ad_c = attp.tile([BC, S], F32, tag="adc", name="ad_c")
                    nc.scalar.activation(ad_c[:], scores_c[:], ACTF.Exp,
                                         bias=negmx[:, :1], accum_out=ssum[:, :1])
                    srec = attp.tile([BC, 1], F32, tag="srec", name="srec")
                    nc.vector.reciprocal(srec[:], ssum[:])
                    nc.vector.tensor_scalar(ad_c[:], ad_c[:], srec[:, :1], None,
                                            op0=ALU.mult)
                    nc.scalar.dma_start(ad_o.ap()[bsl, :], ad_c[:])
                    ad16_c = attp.tile([BC, S], FP16, tag="ad16", name="ad16_c")
                    nc.vector.tensor_copy(ad16_c[:], ad_c[:])
                    nc.scalar.dma_start(ad_dram[bsl, :], ad16_c[:])

                    # context: ctxT[h, b] = sum_s att_dist[b, s] * encT[h, b, s]
                    adf = attp.tile([1, CSZ], FP16, tag="adf", name="adf")
                    nc.scalar.dma_start(
                        adf[:1, :],
                        ad_dram[bsl, :].rearrange("b s -> (b s)").unsqueeze(0))
                    arep = attp.tile([P, CSZ], FP16, tag="arep", name="arep")
                    nc.gpsimd.partition_broadcast(arep[:], adf[:1, :])
                    for t in range(HT):
                        prod = attp.tile([P, CSZ], FP16, tag="prod", name="prod")
                        nc.vector.tensor_mul(prod[:], enc_c[:, t, :], arep[:])
                        nc.vector.tensor_reduce(
                            ctxT_f[:, t, bsl],
                            prod[:].rearrange("p (b s) -> p b s", b=BC),
                            axis=AX.X, op=ALU.add)

            nc.vector.tensor_copy(ctxT_sb[:], ctxT_f[:])

            # ---------------- dec_out, p_gen ----------------
            lhs_list = [hnT_sb[:, 0, :], hnT_sb[:, 1, :],
                        ctxT_sb[:, 0, :], ctxT_sb[:, 1, :]]
            od_ps = mm_ps.tile([P, VCH], F32, tag="mm", name="od_ps")
            for kt in range(4):
                nc.tensor.matmul(od_ps[:, :H], lhs_list[kt], outhW_sb[:, kt, :],
                                 start=(kt == 0), stop=(kt == 3))
            dec_sb = sb.tile([P, H], F32)
            nc.vector.tensor_add(dec_sb[:], od_ps[:, :H], outhb_rep[:])

            pg_ps = mm_ps.tile([P, VCH], F32, tag="mm", name="pg_ps")
            for i, lhs in enumerate(lhs_list + [xT_sb[:]]):
                nc.tensor.matmul(pg_ps[:, :1], lhs.bitcast(F32),
                                 genw_sb[:, i:i + 1].bitcast(F32),
                                 start=(i == 0), stop=(i == 4))
            # sigmoid(x) = 1 / (1 + exp(-x)); keeps ACT on the Exp table
            pg_sb = sb.tile([P, 1], F32)
            neggenb = sb.tile([P, 1], F32)
            nc.vector.tensor_scalar(neggenb[:], genb_pp[:], -1.0, None,
                                    op0=ALU.mult)
            nc.scalar.activation(pg_sb[:], pg_ps[:, :1], ACTF.Exp,
                                 scale=-1.0, bias=neggenb[:, :1])
            nc.vector.tensor_scalar(pg_sb[:], pg_sb[:], 1.0, None, op0=ALU.add)
            nc.vector.reciprocal(pg_sb[:], pg_sb[:])
            nc.scalar.dma_start(pg_o.ap(), pg_sb[:])

            # decT (fp16 + fp16 residual) -> allgather
            decT_sb = sb.tile([P, HT, 2, P], FP16)
            for t in range(HT):
                psT = tp_ps.tile([P, P], F32, tag="tp", name="tp")
                nc.tensor.transpose(psT[:], dec_sb[:, t * P:(t + 1) * P],
                                    ident[:])
                nc.vector.tensor_copy(decT_sb[:, t, 0, :], psT[:])
                res_f = sb.tile([P, P], F32, tag="resf", name="res_f")
                nc.vector.tensor_sub(res_f[:], psT[:], decT_sb[:, t, 0, :])
                nc.vector.tensor_copy(decT_sb[:, t, 1, :], res_f[:])
            nc.sync.dma_start(ag_in[:].rearrange("(t u p) b -> p t u b",
                                                 t=HT, u=2),
                              decT_sb[:])

            # keep the PE HAM-warm through the AllGather wait: a chain of
            # bf16 matmuls gated on decT (they run during the collective)
            warm_lhs = const.tile([P, P], BF16)
            nc.vector.tensor_copy(warm_lhs[:], decT_sb[:, 0, :].bitcast(F32))
            warm_rhs = const.tile([P, VCH], BF16)
            nc.vector.tensor_copy(warm_rhs[:],
                                  warm_lhs[:, :1].broadcast_to([P, VCH]))
            warm_ps = tp_ps.tile([P, VCH], F32, tag="warm", name="warm_ps",
                                 bufs=1)
            for i in range(250):
                nc.tensor.matmul(warm_ps[:], warm_lhs[:], warm_rhs[:],
                                 start=True, stop=True)
            warm_out = const.tile([P, 32], F32)
            nc.vector.tensor_copy(warm_out[:], warm_ps[:, :32])
        # `w` pool (and enc) closed: SBUF free for vocab phase

        nc.gpsimd.collective_compute(
            "AllGather", ALU.bypass,
            replica_groups=[list(range(NC))],
            ins=[ag_in[:].opt()], outs=[ag_out[:].opt()])
        ag_view = ag_out[:].rearrange("(c t u p) b -> p t u c b", c=NC, t=HT,
                                      u=2)
        for t in range(HT):
            for u in range(2):
                nc.scalar.dma_start(
                    decTall[:, t, u, :].rearrange("p (c b) -> p c b", c=NC),
                    ag_view[:, t, u, :, :])

        # ---------------- vocab matmul + softmax (vocab-sharded) ----------------
        GROUPS = [[0, 1, 2, 3, 4], [5, 6, 7]]
        with tc.tile_pool(name="vocab", bufs=1) as vb, \
             tc.tile_pool(name="expp", bufs=1) as expp, \
             tc.tile_pool(name="vsmall", bufs=3) as vsm:
            wv_sb = vb.tile([P, HT, VL], FP16)
            wsrc = wvT_t.ap().rearrange("(t p) v -> p t v", t=HT)
            for c in range(4):
                cs = slice(c * VL // 4, (c + 1) * VL // 4)
                nc.sync.dma_start(wv_sb[:, :, cs], wsrc[:, :, cs])
            bv_sb = vb.tile([1, VL], FP16)
            nc.sync.dma_start(bv_sb[:1, :], bv_t.ap())
            bvrep = vb.tile([P, VL], FP16)
            nc.gpsimd.partition_broadcast(bvrep[:], bv_sb[:1, :])
            inv128 = vb.tile([P, P], FP16)
            nc.vector.memset(inv128[:], 1.0 / P)

            # bridge dummies: keep PE warm across the decTall load
            for i in range(15):
                wp2 = tp_ps.tile([P, VCH], F32, tag="warm", name="warm2",
                                 bufs=1)
                nc.tensor.matmul(wp2[:], decTall[:, 0, 0, :P],
                                 wv_sb[:, 0, :VCH], start=True, stop=True)

            dens = vb.tile([P, NC], F32)
            recs = vb.tile([P, NC], F32)
            expbs = [expp.tile([P, VL], BF16, name=f"expb{r}") for r in range(NC)]
            gden_in = [dram.tile([P, len(g)], F32, name=f"gden_in{gi}")
                       for gi, g in enumerate(GROUPS)]
            gden_out = [dram.tile([P, len(g)], F32, name=f"gden_out{gi}")
                        for gi, g in enumerate(GROUPS)]

            def compute_r(r):
                expb = expbs[r]
                parts = vsm.tile([P, NVCH], F32, tag="parts", name="parts")
                lhs = [decTall[:, t, u, r * P:(r + 1) * P]
                       for t in range(HT) for u in range(2)]
                for ch in range(NVCH):
                    c0 = ch * VCH
                    cw = min(VCH, VL - c0)
                    ps = mm_ps.tile([P, VCH], F32, tag="mm", name="vps")
                    nc.tensor.matmul(ps[:, :cw], lhs[0], wv_sb[:, 0, c0:c0 + cw],
                                     start=True, stop=False)
                    nc.tensor.matmul(ps[:, :cw], lhs[2], wv_sb[:, 1, c0:c0 + cw],
                                     start=False, stop=False)
                    nc.tensor.matmul(ps[:, :cw], lhs[1], wv_sb[:, 0, c0:c0 + cw],
                                     start=False, stop=False)
                    nc.tensor.matmul(ps[:, :cw], lhs[3], wv_sb[:, 1, c0:c0 + cw],
                                     start=False, stop=False)
                    nc.tensor.matmul(ps[:, :cw], inv128[:],
                                     bvrep[:, c0:c0 + cw],
                                     start=False, stop=True)
                    nc.scalar.activation(expb[:, c0:c0 + cw], ps[:, :cw],
                                         ACTF.Exp,
                                         accum_out=parts[:, ch:ch + 1])
                nc.vector.tensor_reduce(dens[:, r:r + 1], parts[:], axis=AX.X,
                                        op=ALU.add)

            def norm_group(gi):
                g = GROUPS[gi]
                dsum = vsm.tile([P, len(g)], F32, tag="dsum", name="dsum")
                nc.scalar.dma_start(dsum[:, :len(g)], gden_out[gi][:])
                nc.vector.reciprocal(recs[:, g[0]:g[-1] + 1], dsum[:, :len(g)])
                for r in g:
                    for ch in range(NVCH):
                        c0 = ch * VCH
                        cw = min(VCH, VL - c0)
                        nrm = vsm.tile([P, VCH], F32, tag="nrm", name="nrm",
                                       bufs=4)
                        nc.vector.tensor_scalar(nrm[:, :cw],
                                                expbs[r][:, c0:c0 + cw],
                                                recs[:, r:r + 1], None,
                                                op0=ALU.mult)
                        eng = nc.scalar if ch % 2 == 0 else nc.sync
                        eng.dma_start(
                            pv_o.ap()[r * P:(r + 1) * P, c0:c0 + cw],
                            nrm[:, :cw])

            for gi, g in enumerate(GROUPS):
                for j, r in enumerate(g):
                    compute_r(r)
                    # overlap the previous group's normalize + output DMA
                    # with this group's compute (after the AR has landed)
                    if gi > 0 and j == 1:
                        norm_group(gi - 1)
                nc.sync.dma_start(gden_in[gi][:], dens[:, g[0]:g[-1] + 1])
                nc.gpsimd.collective_compute(
                    "AllReduce", ALU.add,
                    replica_groups=[list(range(NC))],
                    ins=[gden_in[gi][:].opt()], outs=[gden_out[gi][:].opt()])
            norm_group(len(GROUPS) - 1)

    nc.compile()
    return nc


def _prep_inputs(input_token, last_decoder_hidden, encoder_states, emb,
                 W_ih, W_hh, b_ih, b_hh, w_h, w_s, att_bias, attn_v,
                 gen_W, gen_b, outh_W, outh_b, outv_W, outv_b):
    f = np.float32
    emb = np.ascontiguousarray(emb, dtype=f)
    wihT = np.ascontiguousarray(np.asarray(W_ih, f).T)               # [128, 768]
    whhT = np.ascontiguousarray(np.asarray(W_hh, f).T).reshape(HT, P, G)
    outhWT = np.ascontiguousarray(np.asarray(outh_W, f).T).reshape(4, P, H)
    wh2 = np.ascontiguousarray(np.asarray(w_h, f).reshape(HT, P).T)  # [128, 2]
    ws2 = np.ascontiguousarray(np.asarray(w_s, f).reshape(HT, P).T)
    av2 = np.ascontiguousarray(np.asarray(attn_v, f).reshape(HT, P).T)
    genw = np.ascontiguousarray(np.asarray(gen_W, f).reshape(5, P).T)  # [128, 5]
    bih = np.asarray(b_ih, f).reshape(1, G)
    bhh = np.asarray(b_hh, f).reshape(1, G)
    outhb = np.asarray(outh_b, f).reshape(1, H)
    attb = np.asarray(att_bias, f).reshape(1, 1)
    genb = np.asarray(gen_b, f).reshape(1, 1)
    outvT = np.ascontiguousarray(np.asarray(outv_W, f).T).astype(np.float16)
    outvb = np.asarray(outv_b, f).reshape(1, V).astype(np.float16)
    tok_all = np.asarray(input_token).astype(np.int32).reshape(B, 1)
    h_all = np.asarray(last_decoder_hidden, f)
    enc_all = np.asarray(encoder_states, f)

    in_maps = []
    for c in range(NC):
        bs = slice(c * BL, (c + 1) * BL)
        vs = slice(c * VL, (c + 1) * VL)
        encT = np.ascontiguousarray(
            enc_all[bs].transpose(2, 0, 1)).astype(np.float16)  # [H, BL, S]
        in_maps.append({
            "tok": tok_all[bs], "emb": emb, "h0": np.ascontiguousarray(h_all[bs]),
            "encT": encT, "wihT": wihT, "whhT": whhT, "bih": bih, "bhh": bhh,
            "wh2": wh2, "ws2": ws2, "av2": av2, "attb": attb,
            "genw": genw, "genb": genb, "outhWT": outhWT, "outhb": outhb,
            "wvT": np.ascontiguousarray(outvT[:, vs]),
            "bv": np.ascontiguousarray(outvb[:, vs]),
        })
    return in_maps


def _assemble(results):
    hn = np.concatenate([r["hn_o"] for r in results], axis=0)        # [B, H]
    pg = np.concatenate([r["pg_o"] for r in results], axis=0)        # [B, 1]
    ad = np.concatenate([r["ad_o"] for r in results], axis=0)        # [B, S]
    pv = np.concatenate([r["pv_o"] for r in results], axis=1)        # [B, V]
    return hn[None], pg, pv, ad


def _run(in_maps, trace=False, tmpdir=None):
    if "nc" not in _CACHE:
        _CACHE["nc"] = build_bass()
    kw = {}
    if trace:
        kw = {"trace": True, "tmpdir": tmpdir}
    res = bass_utils.run_bass_kernel_spmd(
        _CACHE["nc"], in_maps, core_ids=list(range(NC)), **kw)
    return res


def kernel(**inputs):
    in_maps = _prep_inputs(**inputs)
    res = _run(in_maps)
    return _assemble(res.results)


def kernel_traced(tmpdir, **inputs):
    """Like kernel() but returns (outputs, BassKernelResults) with NTFF profile."""
    in_maps = _prep_inputs(**inputs)
    res = _run(in_maps, trace=True, tmpdir=tmpdir)
    return _assemble(res.results), res


# revision 36
# speedup vs baseline: 1.0002x; 1.0002x over previous
"""Trainium2 Bass kernel for nn_AttnDecoderRNN (B=1024, S=100, H=256, E=128, V=50000).

Sharding across 8 NeuronCores:
  - batch-parallel (128 rows/core) for embedding gather, GRU cell, additive
    attention, out-hidden projection, p_gen;
  - vocab-parallel (6250 cols/core) for the vocab matmul + softmax:
    AllGather of the transposed decoder output, per-row-tile AllReduce of
    softmax denominators.

Self-contained: hardcodes all shapes; imports only concourse + numpy.
"""
import numpy as np
from contextlib import ExitStack

import concourse.bass as bass
import concourse.bacc as bacc
import concourse.tile as tile
import concourse.mybir as mybir
from concourse import bass_utils
from concourse.masks import make_identity

F32 = mybir.dt.float32
F32R = mybir.dt.float32r
BF16 = mybir.dt.bfloat16
FP16 = mybir.dt.float16
I32 = mybir.dt.int32
AX = mybir.AxisListType
ALU = mybir.AluOpType
ACTF = mybir.ActivationFunctionType

NC = 8           # cores
B = 1024         # batch
BL = B // NC     # batch rows per core (=128)
S = 100          # encoder length
H = 256          # hidden
E = 128          # embedding dim
V = 50000        # vocab
VL = V // NC     # vocab cols per core (=6250)
P = 128          # partitions
HT = H // P      # h tiles (=2)
G = 3 * H        # gru gate width (=768)

BC = 16          # attention batch chunk
NBC = BL // BC   # = 8
SCN = 4 * S      # scores matmul N-chunk (4 batch rows = 400)
VCH = 512        # vocab matmul N chunk
NVCH = (VL + VCH - 1) // VCH   # = 13 (12x512 + 106)

_CACHE = {}


def build_bass():
    nc = bacc.Bacc("TRN2", target_bir_lowering=False, debug=False, num_devices=NC)

    # ---------------- I/O ----------------
    tok = nc.dram_tensor("tok", [BL, 1], I32, kind="ExternalInput")
    emb_t = nc.dram_tensor("emb", [V, E], F32, kind="ExternalInput")
    h_t = nc.dram_tensor("h0", [BL, H], F32, kind="ExternalInput")
    encT_t = nc.dram_tensor("encT", [H, BL, S], FP16, kind="ExternalInput")
    wihT_t = nc.dram_tensor("wihT", [E, G], F32R, kind="ExternalInput")
    whhT_t = nc.dram_tensor("whhT", [HT, P, G], F32R, kind="ExternalInput")
    bih_t = nc.dram_tensor("bih", [1, G], F32, kind="ExternalInput")
    bhh_t = nc.dram_tensor("bhh", [1, G], F32, kind="ExternalInput")
    wh2_t = nc.dram_tensor("wh2", [P, HT], F32, kind="ExternalInput")
    ws2_t = nc.dram_tensor("ws2", [P, HT], F32, kind="ExternalInput")
    av2_t = nc.dram_tensor("av2", [P, HT], F32, kind="ExternalInput")
    attb_t = nc.dram_tensor("attb", [1, 1], F32, kind="ExternalInput")
    genw_t = nc.dram_tensor("genw", [P, 5], F32R, kind="ExternalInput")
    genb_t = nc.dram_tensor("genb", [1, 1], F32, kind="ExternalInput")
    outhWT_t = nc.dram_tensor("outhWT", [4, P, H], F32R, kind="ExternalInput")
    outhb_t = nc.dram_tensor("outhb", [1, H], F32, kind="ExternalInput")
    wvT_t = nc.dram_tensor("wvT", [H, VL], FP16, kind="ExternalInput")
    bv_t = nc.dram_tensor("bv", [1, VL], FP16, kind="ExternalInput")

    hn_o = nc.dram_tensor("hn_o", [BL, H], F32, kind="ExternalOutput")
    pg_o = nc.dram_tensor("pg_o", [BL, 1], F32, kind="ExternalOutput")
    ad_o = nc.dram_tensor("ad_o", [BL, S], F32, kind="ExternalOutput")
    pv_o = nc.dram_tensor("pv_o", [B, VL], F32, kind="ExternalOutput")

    with tile.TileContext(nc) as tc, ExitStack() as ctx:
        const = ctx.enter_context(tc.tile_pool(name="const", bufs=1))
        tp_ps = ctx.enter_context(tc.tile_pool(name="tp_ps", bufs=1, space="PSUM"))
        mm_ps = ctx.enter_context(tc.tile_pool(name="mm_ps", bufs=6, space="PSUM"))
        dram = ctx.enter_context(tc.tile_pool(name="dram", bufs=1, space="DRAM"))

        ident = const.tile([P, P], F32)
        make_identity(nc, ident[:])

        def transpose128(dst_ap, src_ap):
            """dst[j,i] = src[i,j] for a [128,128] block (via PE + DVE copy)."""
            ps = tp_ps.tile([P, P], F32, tag="tp", name="tp")
            nc.tensor.transpose(ps[:], src_ap, ident[:])
            nc.vector.tensor_copy(dst_ap, ps[:])

        # DRAM bounce buffers
        sc_dram = dram.tile([1, BL * S], F32, name="sc_dram")
        ad_dram = dram.tile([BL, S], FP16, name="ad_dram")
        ag_in = dram.tile([HT * 2 * P, P], FP16, name="ag_in")
        ag_out = dram.tile([NC * HT * 2 * P, P], FP16, name="ag_out")
        den_in = [dram.tile([P, 1], F32, name=f"den_in{r}") for r in range(NC)]
        den_out = [dram.tile([P, 1], F32, name=f"den_out{r}") for r in range(NC)]

        # tiles that must survive into the vocab phase
        decTall = const.tile([P, HT, 2, NC * P], FP16)
        ones_r = const.tile([1, P], F32R)

        with tc.tile_pool(name="w", bufs=1) as sb:
            # ---------------- prologue loads ----------------
            tok_sb = sb.tile([P, 1], I32)
            nc.sync.dma_start(tok_sb[:], tok.ap())
            x_sb = sb.tile([P, E], F32)
            nc.gpsimd.indirect_dma_start(
                out=x_sb[:], out_offset=None, in_=emb_t.ap(),
                in_offset=bass.IndirectOffsetOnAxis(ap=tok_sb[:, :1], axis=0))

            h_sb = sb.tile([P, H], F32)
            nc.sync.dma_start(h_sb[:], h_t.ap())
            wih_sb = sb.tile([P, G], F32R)
            nc.sync.dma_start(wih_sb[:], wihT_t.ap())
            whh_sb = sb.tile([P, HT, G], F32R)
            nc.sync.dma_start(whh_sb[:], whhT_t.ap().rearrange("t p n -> p t n"))
            wh2_sb = sb.tile([P, HT], F32)
            nc.sync.dma_start(wh2_sb[:], wh2_t.ap())
            ws2_sb = sb.tile([P, HT], F32)
            nc.sync.dma_start(ws2_sb[:], ws2_t.ap())
            av2_sb = sb.tile([P, HT], F32)
            nc.sync.dma_start(av2_sb[:], av2_t.ap())
            genw_sb = sb.tile([P, 5], F32R)
            nc.sync.dma_start(genw_sb[:], genw_t.ap())
            outhW_sb = sb.tile([P, 4, H], F32R)
            nc.sync.dma_start(outhW_sb[:], outhWT_t.ap().rearrange("t p n -> p t n"))

            NR = 2 * G + H + 2
            gru_ctx = ExitStack()
            grup = gru_ctx.enter_context(tc.tile_pool(name="grup", bufs=1))
            rows_sb = grup.tile([1, NR], F32)  # bih | bhh | outhb | attb | genb
            nc.sync.dma_start(rows_sb[:1, 0:G], bih_t.ap())
            nc.sync.dma_start(rows_sb[:1, G:2 * G], bhh_t.ap())
            nc.sync.dma_start(rows_sb[:1, 2 * G:2 * G + H], outhb_t.ap())
            nc.sync.dma_start(rows_sb[:1, 2 * G + H:2 * G + H + 1], attb_t.ap())
            nc.sync.dma_start(rows_sb[:1, 2 * G + H + 1:NR], genb_t.ap())

            ones_f = grup.tile([1, P], F32)
            nc.vector.memset(ones_f[:], 1.0)
            nc.vector.tensor_copy(ones_r[:], ones_f[:])

            # replicated bias tiles
            brz_row = grup.tile([1, 2 * H], F32)   # (b_ih + b_hh)[0:512]
            nc.vector.tensor_add(brz_row[:1, :], rows_sb[:1, 0:2 * H],
                                 rows_sb[:1, G:G + 2 * H])
            brz_rep = grup.tile([P, 2 * H], F32)
            nc.gpsimd.partition_broadcast(brz_rep[:], brz_row[:1, :])
            bihn_rep = grup.tile([P, H], F32)
            nc.gpsimd.partition_broadcast(bihn_rep[:], rows_sb[:1, 2 * H:G])
            bhhn_rep = grup.tile([P, H], F32)
            nc.gpsimd.partition_broadcast(bhhn_rep[:], rows_sb[:1, G + 2 * H:2 * G])
            outhb_rep = sb.tile([P, H], F32)
            nc.gpsimd.partition_broadcast(outhb_rep[:], rows_sb[:1, 2 * G:2 * G + H])
            attb_pp = sb.tile([P, 1], F32)
            nc.gpsimd.partition_broadcast(attb_pp[:],
                                          rows_sb[:1, 2 * G + H:2 * G + H + 1])
            genb_pp = sb.tile([P, 1], F32)
            nc.gpsimd.partition_broadcast(genb_pp[:], rows_sb[:1, 2 * G + H + 1:NR])

            # attn_v replicated into lhsT form [P(h), HT, 128(m)]
            av_rep = sb.tile([P, HT, P], FP16)
            for t in range(HT):
                nc.vector.tensor_copy(av_rep[:, t, :],
                                      av2_sb[:, t:t + 1].broadcast_to([P, P]))

            # ---------------- GRU ----------------
            xT_sb = sb.tile([P, E], F32R)
            transpose128(xT_sb[:], x_sb[:])
            hT_sb = sb.tile([P, HT, P], F32R)
            for t in range(HT):
                transpose128(hT_sb[:, t, :], h_sb[:, t * P:(t + 1) * P])

            hn_sb = grup.tile([P, H], F32)
            # gate chunks: a = cols [0:512] (r,z), b = cols [512:768] (n)
            gi_a = mm_ps.tile([P, VCH], F32, tag="mm", name="gi_a")
            gi_b = mm_ps.tile([P, VCH], F32, tag="mm", name="gi_b")
            gh_a = mm_ps.tile([P, VCH], F32, tag="mm", name="gh_a")
            gh_b = mm_ps.tile([P, VCH], F32, tag="mm", name="gh_b")
            for (pi, ph, n0, n1) in [(gi_a, gh_a, 0, 512), (gi_b, gh_b, 512, G)]:
                w = n1 - n0
                nc.tensor.matmul(pi[:, :w], xT_sb[:], wih_sb[:, n0:n1],
                                 start=True, stop=True)
                for t in range(HT):
                    nc.tensor.matmul(ph[:, :w], hT_sb[:, t, :],
                                     whh_sb[:, t, n0:n1],
                                     start=(t == 0), stop=(t == HT - 1))

            # r,z = sigmoid(gi + gh + bih + bhh) over [0:512]
            rz_sb = grup.tile([P, 2 * H], F32)
            nc.scalar.copy(rz_sb[:], gi_a[:, :2 * H])
            nc.vector.tensor_add(rz_sb[:], rz_sb[:], gh_a[:, :2 * H])
            nc.vector.tensor_add(rz_sb[:], rz_sb[:], brz_rep[:])
            # sigmoid(x) = 0.5*tanh(0.5*x) + 0.5 (keeps ACT on the Tanh table)
            nc.scalar.activation(rz_sb[:], rz_sb[:], ACTF.Tanh, scale=0.5)
            nc.vector.tensor_scalar(rz_sb[:], rz_sb[:], 0.5, 0.5,
                                    op0=ALU.mult, op1=ALU.add)
            # n = tanh(i_n + b_ihn + r*(h_n + b_hhn))
            n_sb = grup.tile([P, H], F32)
            nc.vector.tensor_add(n_sb[:], gh_b[:, :H], bhhn_rep[:])
            nc.vector.tensor_mul(n_sb[:], n_sb[:], rz_sb[:, 0:H])
            nc.vector.tensor_add(n_sb[:], n_sb[:], gi_b[:, :H])
            nc.vector.tensor_add(n_sb[:], n_sb[:], bihn_rep[:])
            nc.scalar.activation(n_sb[:], n_sb[:], ACTF.Tanh)
            # h_new = n + z*(h - n)
            nc.vector.tensor_sub(hn_sb[:], h_sb[:], n_sb[:])
            nc.vector.tensor_mul(hn_sb[:], hn_sb[:], rz_sb[:, H:2 * H])
            nc.vector.tensor_add(hn_sb[:], hn_sb[:], n_sb[:])
            nc.scalar.dma_start(hn_o.ap(), hn_sb[:])

            hnT_sb = sb.tile([P, HT, P], F32R)
            for t in range(HT):
                transpose128(hnT_sb[:, t, :], hn_sb[:, t * P:(t + 1) * P])
            # u'_T[h, b] = (w_s/w_h)*h_newT + att_bias/w_h so that
            # tanh(w_h*E + u) == tanh(w_h*(E + u')): w_h moves into ACT scale
            rwh2 = sb.tile([P, HT], F32)
            nc.vector.reciprocal(rwh2[:], wh2_sb[:])
            wsw = sb.tile([P, HT], F32)
            nc.vector.tensor_mul(wsw[:], ws2_sb[:], rwh2[:])
            abw = sb.tile([P, HT], F32)
            nc.vector.tensor_mul(abw[:], attb_pp[:, :1].broadcast_to([P, HT]),
                                 rwh2[:])
            uT_sb = sb.tile([P, HT, P], FP16)
            for t in range(HT):
                nc.vector.tensor_scalar(uT_sb[:, t, :], hnT_sb[:, t, :],
                                        wsw[:, t:t + 1], abw[:, t:t + 1],
                                        op0=ALU.mult, op1=ALU.add)

            gru_ctx.close()

            # ---------------- attention ----------------
            # processed in NBC independent batch chunks of BC rows:
            # scores -> per-chunk softmax -> context, streaming encoder chunks
            ctxT_f = sb.tile([P, HT, P], F32)
            ctxT_sb = sb.tile([P, HT, P], F32R)
            CSZ = BC * S  # 1600

            with tc.tile_pool(name="enc", bufs=8) as encp, \
                 tc.tile_pool(name="attw", bufs=3) as attp:
                esrc = encT_t.ap().rearrange("(t p) b s -> p t (b s)", t=HT)
                enc_tiles = []
                for c in range(NBC):
                    cs = slice(c * CSZ, (c + 1) * CSZ)
                    enc_c = encp.tile([P, HT, CSZ], FP16, tag="encc",
                                      name=f"enc_c{c}")
                    nc.sync.dma_start(enc_c[:], esrc[:, :, cs])
                    enc_tiles.append(enc_c)
                for c in range(NBC):
                    bsl = slice(c * BC, (c + 1) * BC)
                    enc_c = enc_tiles[c]
                    att_c = attp.tile([P, HT, CSZ], FP16, tag="attc", name="att_c")
                    for t in range(HT):
                        # att = tanh(w_h*(E + u'))  (u' broadcast over s;
                        # w_h applied via the per-partition ACT scale)
                        nc.vector.tensor_tensor(
                            att_c[:, t, :].rearrange("p (b s) -> p b s", b=BC),
                            enc_c[:, t, :].rearrange("p (b s) -> p b s", b=BC),
                            uT_sb[:, t, bsl].unsqueeze(2)
                                .broadcast_to([P, BC, S]),
                            op=ALU.add)
                        nc.scalar.activation(att_c[:, t, :], att_c[:, t, :],
                                             ACTF.Tanh,
                                             scale=wh2_sb[:, t:t + 1])
                    # scores chunk: av . att -> rows replicated; keep row 0
                    for k, n0 in enumerate(range(0, CSZ, SCN)):
                        sc_ps = mm_ps.tile([P, VCH], F32, tag="mm", name="sc_ps")
                        for t in range(HT):
                            nc.tensor.matmul(sc_ps[:, :SCN], av_rep[:, t, :],
                                             att_c[:, t, n0:n0 + SCN],
                                             start=(t == 0), stop=(t == HT - 1))
                        sc_row = attp.tile([1, SCN], F32, tag="scrow", name="sc_row")
                        if k % 2:
                            nc.vector.tensor_copy(sc_row[:1, :], sc_ps[:1, :SCN])
                        else:
                            nc.scalar.copy(sc_row[:1, :], sc_ps[:1, :SCN])
                        off = c * CSZ + n0
                        nc.scalar.dma_start(sc_dram[:1, off:off + SCN],
                                            sc_row[:1, :])

                    # scatter scores [1, CSZ] -> [BC, S] and per-chunk softmax
                    scores_c = attp.tile([BC, S], F32, tag="scc", name="scores_c")
                    nc.scalar.dma_start(
                        scores_c[:],
                        sc_dram[:1, c * CSZ:(c + 1) * CSZ]
                            .rearrange("o (b s) -> (o b) s", b=BC))
                    ssum = attp.tile([BC, 1], F32, tag="ssum", name="ssum")
                    ad_c = attp.tile([BC, S], F32, tag="adc", name="ad_c")
                    nc.scalar.activation(ad_c[:], scores_c[:], ACTF.Exp,
                                         accum_out=ssum[:, :1])
                    srec = attp.tile([BC, 1], F32, tag="srec", name="srec")
                    nc.vector.reciprocal(srec[:], ssum[:])
                    nc.vector.tensor_scalar(ad_c[:], ad_c[:], srec[:, :1], None,
                                            op0=ALU.mult)
                    nc.scalar.dma_start(ad_o.ap()[bsl, :], ad_c[:])
                    ad16_c = attp.tile([BC, S], FP16, tag="ad16", name="ad16_c")
                    nc.vector.tensor_copy(ad16_c[:], ad_c[:])
                    nc.scalar.dma_start(ad_dram[bsl, :], ad16_c[:])

                    # context: ctxT[h, b] = sum_s att_dist[b, s] * encT[h, b, s]
                    adf = attp.tile([1, CSZ], FP16, tag="adf", name="adf")
                    nc.scalar.dma_start(
                        adf[:1, :],
                        ad_dram[bsl, :].rearrange("b s -> (b s)").unsqueeze(0))
                    arep = attp.tile([P, CSZ], FP16, tag="arep", name="arep")
                    nc.gpsimd.partition_broadcast(arep[:], adf[:1, :])
                    prod = attp.tile([P, HT, CSZ], FP16, tag="prod", name="prod")
                    nc.vector.tensor_mul(
                        prod[:], enc_c[:],
                        arep[:].unsqueeze(1).broadcast_to([P, HT, CSZ]))
                    nc.vector.tensor_reduce(
                        ctxT_f[:, :, bsl],
                        prod[:].rearrange("p t (b s) -> p t b s", b=BC),
                        axis=AX.X, op=ALU.add)

            nc.vector.tensor_copy(ctxT_sb[:], ctxT_f[:])

            # ---------------- dec_out, p_gen ----------------
            lhs_list = [hnT_sb[:, 0, :], hnT_sb[:, 1, :],
                        ctxT_sb[:, 0, :], ctxT_sb[:, 1, :]]
            od_ps = mm_ps.tile([P, VCH], F32, tag="mm", name="od_ps")
            for kt in range(4):
                nc.tensor.matmul(od_ps[:, :H], lhs_list[kt], outhW_sb[:, kt, :],
                                 start=(kt == 0), stop=(kt == 3))
            dec_sb = sb.tile([P, H], F32)
            nc.vector.tensor_add(dec_sb[:], od_ps[:, :H], outhb_rep[:])

            pg_ps = mm_ps.tile([P, VCH], F32, tag="mm", name="pg_ps")
            for i, lhs in enumerate(lhs_list + [xT_sb[:]]):
                nc.tensor.matmul(pg_ps[:, :1], lhs.bitcast(F32),
                                 genw_sb[:, i:i + 1].bitcast(F32),
                                 start=(i == 0), stop=(i == 4))
            # sigmoid(x) = 1 / (1 + exp(-x)); keeps ACT on the Exp table
            pg_sb = sb.tile([P, 1], F32)
            neggenb = sb.tile([P, 1], F32)
            nc.vector.tensor_scalar(neggenb[:], genb_pp[:], -1.0, None,
                                    op0=ALU.mult)
            nc.scalar.activation(pg_sb[:], pg_ps[:, :1], ACTF.Exp,
                                 scale=-1.0, bias=neggenb[:, :1])
            nc.vector.tensor_scalar(pg_sb[:], pg_sb[:], 1.0, None, op0=ALU.add)
            nc.vector.reciprocal(pg_sb[:], pg_sb[:])
            nc.scalar.dma_start(pg_o.ap(), pg_sb[:])

            # decT (fp16 + fp16 residual) -> allgather
            decT_sb = sb.tile([P, HT, 2, P], FP16)
            for t in range(HT):
                psT = tp_ps.tile([P, P], F32, tag="tp", name="tp")
                nc.tensor.transpose(psT[:], dec_sb[:, t * P:(t + 1) * P],
                                    ident[:])
                nc.vector.tensor_copy(decT_sb[:, t, 0, :], psT[:])
                res_f = sb.tile([P, P], F32, tag="resf", name="res_f")
                nc.vector.tensor_sub(res_f[:], psT[:], decT_sb[:, t, 0, :])
                nc.vector.tensor_copy(decT_sb[:, t, 1, :], res_f[:])
            nc.sync.dma_start(ag_in[:].rearrange("(t u p) b -> p t u b",
                                                 t=HT, u=2),
                              decT_sb[:])

            # keep the PE HAM-warm through the AllGather wait: a chain of
            # bf16 matmuls gated on decT (they run during the collective)
            warm_lhs = const.tile([P, P], BF16)
            nc.vector.tensor_copy(warm_lhs[:], decT_sb[:, 0, :].bitcast(F32))
            warm_rhs = const.tile([P, VCH], BF16)
            nc.vector.tensor_copy(warm_rhs[:],
                                  warm_lhs[:, :1].broadcast_to([P, VCH]))
            warm_ps = tp_ps.tile([P, VCH], F32, tag="warm", name="warm_ps",
                                 bufs=1)
            for i in range(250):
                nc.tensor.matmul(warm_ps[:], warm_lhs[:], warm_rhs[:],
                                 start=True, stop=True)
            warm_out = const.tile([P, 32], F32)
            nc.vector.tensor_copy(warm_out[:], warm_ps[:, :32])
        # `w` pool (and enc) closed: SBUF free for vocab phase

        nc.gpsimd.collective_compute(
            "AllGather", ALU.bypass,
            replica_groups=[list(range(NC))],
            ins=[ag_in[:].opt()], outs=[ag_out[:].opt()])
        ag_view = ag_out[:].rearrange("(c t u p) b -> p t u c b", c=NC, t=HT,
                                      u=2)
        for t in range(HT):
            for u in range(2):
                nc.scalar.dma_start(
                    decTall[:, t, u, :].rearrange("p (c b) -> p c b", c=NC),
                    ag_view[:, t, u, :, :])

        # ---------------- vocab matmul + softmax (vocab-sharded) ----------------
        GROUPS = [[0, 1, 2, 3, 4], [5, 6, 7]]
        with tc.tile_pool(name="vocab", bufs=1) as vb, \
             tc.tile_pool(name="expp", bufs=1) as expp, \
             tc.tile_pool(name="vsmall", bufs=3) as vsm:
            wv_sb = vb.tile([P, HT, VL], FP16)
            wsrc = wvT_t.ap().rearrange("(t p) v -> p t v", t=HT)
            for c in range(4):
                cs = slice(c * VL // 4, (c + 1) * VL // 4)
                nc.sync.dma_start(wv_sb[:, :, cs], wsrc[:, :, cs])
            bv_sb = vb.tile([1, VL], FP16)
            nc.sync.dma_start(bv_sb[:1, :], bv_t.ap())
            bvrep = vb.tile([P, VL], FP16)
            nc.gpsimd.partition_broadcast(bvrep[:], bv_sb[:1, :])
            inv128 = vb.tile([P, P], FP16)
            nc.vector.memset(inv128[:], 1.0 / P)

            # bridge dummies: keep PE warm across the decTall load
            for i in range(15):
                wp2 = tp_ps.tile([P, VCH], F32, tag="warm", name="warm2",
                                 bufs=1)
                nc.tensor.matmul(wp2[:], decTall[:, 0, 0, :P],
                                 wv_sb[:, 0, :VCH], start=True, stop=True)

            dens = vb.tile([P, NC], F32)
            recs = vb.tile([P, NC], F32)
            expbs = [expp.tile([P, VL], FP16, name=f"expb{r}") for r in range(NC)]
            gden_in = [dram.tile([P, len(g)], F32, name=f"gden_in{gi}")
                       for gi, g in enumerate(GROUPS)]
            gden_out = [dram.tile([P, len(g)], F32, name=f"gden_out{gi}")
                        for gi, g in enumerate(GROUPS)]

            def compute_r(r):
                expb = expbs[r]
                parts = vsm.tile([P, NVCH], F32, tag="parts", name="parts")
                lhs = [decTall[:, t, u, r * P:(r + 1) * P]
                       for t in range(HT) for u in range(2)]
                for ch in range(NVCH):
                    c0 = ch * VCH
                    cw = min(VCH, VL - c0)
                    ps = mm_ps.tile([P, VCH], F32, tag="mm", name="vps")
                    nc.tensor.matmul(ps[:, :cw], lhs[0], wv_sb[:, 0, c0:c0 + cw],
                                     start=True, stop=False)
                    nc.tensor.matmul(ps[:, :cw], lhs[2], wv_sb[:, 1, c0:c0 + cw],
                                     start=False, stop=False)
                    nc.tensor.matmul(ps[:, :cw], lhs[1], wv_sb[:, 0, c0:c0 + cw],
                                     start=False, stop=False)
                    nc.tensor.matmul(ps[:, :cw], lhs[3], wv_sb[:, 1, c0:c0 + cw],
                                     start=False, stop=False)
                    nc.tensor.matmul(ps[:, :cw], inv128[:],
                                     bvrep[:, c0:c0 + cw],
                                     start=False, stop=True)
                    nc.scalar.activation(expb[:, c0:c0 + cw], ps[:, :cw],
                                         ACTF.Exp,
                                         accum_out=parts[:, ch:ch + 1])
                nc.vector.tensor_reduce(dens[:, r:r + 1], parts[:], axis=AX.X,
                                        op=ALU.add)

            def norm_group(gi):
                g = GROUPS[gi]
                dsum = vsm.tile([P, len(g)], F32, tag="dsum", name="dsum")
                nc.scalar.dma_start(dsum[:, :len(g)], gden_out[gi][:])
                nc.vector.reciprocal(recs[:, g[0]:g[-1] + 1], dsum[:, :len(g)])
                for r in g:
                    for ch in range(NVCH):
                        c0 = ch * VCH
                        cw = min(VCH, VL - c0)
                        nrm = vsm.tile([P, VCH], F32, tag="nrm", name="nrm",
                                       bufs=4)
                        nc.vector.tensor_scalar(nrm[:, :cw],
                                                expbs[r][:, c0:c0 + cw],
                                                recs[:, r:r + 1], None,
                                                op0=ALU.mult)
                        eng = nc.scalar if ch % 2 == 0 else nc.sync
                        eng.dma_start(
                            pv_o.ap()[r * P:(r + 1) * P, c0:c0 + cw],
                            nrm[:, :cw])

            for gi, g in enumerate(GROUPS):
                for j, r in enumerate(g):
                    compute_r(r)
                    # overlap the previous group's normalize + output DMA
                    # with this group's compute (after the AR has landed)
                    if gi > 0 and j == 1:
                        norm_group(gi - 1)
                nc.sync.dma_start(gden_in[gi][:], dens[:, g[0]:g[-1] + 1])
                nc.gpsimd.collective_compute(
                    "AllReduce", ALU.add,
                    replica_groups=[list(range(NC))],
                    ins=[gden_in[gi][:].opt()], outs=[gden_out[gi][:].opt()])
            norm_group(len(GROUPS) - 1)

    nc.compile()
    return nc


def _prep_inputs(input_token, last_decoder_hidden, encoder_states, emb,
                 W_ih, W_hh, b_ih, b_hh, w_h, w_s, att_bias, attn_v,
                 gen_W, gen_b, outh_W, outh_b, outv_W, outv_b):
    f = np.float32
    emb = np.ascontiguousarray(emb, dtype=f)
    wihT = np.ascontiguousarray(np.asarray(W_ih, f).T)               # [128, 768]
    whhT = np.ascontiguousarray(np.asarray(W_hh, f).T).reshape(HT, P, G)
    outhWT = np.ascontiguousarray(np.asarray(outh_W, f).T).reshape(4, P, H)
    wh2 = np.ascontiguousarray(np.asarray(w_h, f).reshape(HT, P).T)  # [128, 2]
    ws2 = np.ascontiguousarray(np.asarray(w_s, f).reshape(HT, P).T)
    av2 = np.ascontiguousarray(np.asarray(attn_v, f).reshape(HT, P).T)
    genw = np.ascontiguousarray(np.asarray(gen_W, f).reshape(5, P).T)  # [128, 5]
    bih = np.asarray(b_ih, f).reshape(1, G)
    bhh = np.asarray(b_hh, f).reshape(1, G)
    outhb = np.asarray(outh_b, f).reshape(1, H)
    attb = np.asarray(att_bias, f).reshape(1, 1)
    genb = np.asarray(gen_b, f).reshape(1, 1)
    outvT = np.ascontiguousarray(np.asarray(outv_W, f).T).astype(np.float16)
    outvb = np.asarray(outv_b, f).reshape(1, V).astype(np.float16)
    tok_all = np.asarray(input_token).astype(np.int32).reshape(B, 1)
    h_all = np.asarray(last_decoder_hidden, f)
    enc_all = np.asarray(encoder_states, f)

    in_maps = []
    for c in range(NC):
        bs = slice(c * BL, (c + 1) * BL)
        vs = slice(c * VL, (c + 1) * VL)
        encT = np.ascontiguousarray(
            enc_all[bs].transpose(2, 0, 1)).astype(np.float16)  # [H, BL, S]
        in_maps.append({
            "tok": tok_all[bs], "emb": emb, "h0": np.ascontiguousarray(h_all[bs]),
            "encT": encT, "wihT": wihT, "whhT": whhT, "bih": bih, "bhh": bhh,
            "wh2": wh2, "ws2": ws2, "av2": av2, "attb": attb,
            "genw": genw, "genb": genb, "outhWT": outhWT, "outhb": outhb,
            "wvT": np.ascontiguousarray(outvT[:, vs]),
            "bv": np.ascontiguousarray(outvb[:, vs]),
        })
    return in_maps


def _assemble(results):
    hn = np.concatenate([r["hn_o"] for r in results], axis=0)        # [B, H]
    pg = np.concatenate([r["pg_o"] for r in results], axis=0)        # [B, 1]
    ad = np.concatenate([r["ad_o"] for r in results], axis=0)        # [B, S]
    pv = np.concatenate([r["pv_o"] for r in results], axis=1)        # [B, V]
    return hn[None], pg, pv, ad


def _run(in_maps, trace=False, tmpdir=None):
    if "nc" not in _CACHE:
        _CACHE["nc"] = build_bass()
    kw = {}
    if trace:
        kw = {"trace": True, "tmpdir": tmpdir}
    res = bass_utils.run_bass_kernel_spmd(
        _CACHE["nc"], in_maps, core_ids=list(range(NC)), **kw)
    return res


def kernel(**inputs):
    in_maps = _prep_inputs(**inputs)
    res = _run(in_maps)
    return _assemble(res.results)


def kernel_traced(tmpdir, **inputs):
    """Like kernel() but returns (outputs, BassKernelResults) with NTFF profile."""
    in_maps = _prep_inputs(**inputs)
    res = _run(in_maps, trace=True, tmpdir=tmpdir)
    return _assemble(res.results), res


# revision 39
# speedup vs baseline: 1.0150x; 1.0148x over previous
"""Trainium2 Bass kernel for nn_AttnDecoderRNN (B=1024, S=100, H=256, E=128, V=50000).

Sharding across 8 NeuronCores:
  - batch-parallel (128 rows/core) for embedding gather, GRU cell, additive
    attention, out-hidden projection, p_gen;
  - vocab-parallel (6250 cols/core) for the vocab matmul + softmax:
    AllGather of the transposed decoder output, per-row-tile AllReduce of
    softmax denominators.

Self-contained: hardcodes all shapes; imports only concourse + numpy.
"""
import numpy as np
from contextlib import ExitStack

import concourse.bass as bass
import concourse.bacc as bacc
import concourse.tile as tile
import concourse.mybir as mybir
from concourse import bass_utils
from concourse.masks import make_identity

F32 = mybir.dt.float32
F32R = mybir.dt.float32r
BF16 = mybir.dt.bfloat16
FP16 = mybir.dt.float16
I32 = mybir.dt.int32
AX = mybir.AxisListType
ALU = mybir.AluOpType
ACTF = mybir.ActivationFunctionType

NC = 8           # cores
B = 1024         # batch
BL = B // NC     # batch rows per core (=128)
S = 100          # encoder length
H = 256          # hidden
E = 128          # embedding dim
V = 50000        # vocab
VL = V // NC     # vocab cols per core (=6250)
P = 128          # partitions
HT = H // P      # h tiles (=2)
G = 3 * H        # gru gate width (=768)

BC = 16          # attention batch chunk
NBC = BL // BC   # = 8
SCN = 4 * S      # scores matmul N-chunk (4 batch rows = 400)
VCH = 512        # vocab matmul N chunk
NVCH = (VL + VCH - 1) // VCH   # = 13 (12x512 + 106)

_CACHE = {}


def build_bass():
    nc = bacc.Bacc("TRN2", target_bir_lowering=False, debug=False, num_devices=NC)

    # ---------------- I/O ----------------
    tok = nc.dram_tensor("tok", [BL, 1], I32, kind="ExternalInput")
    emb_t = nc.dram_tensor("emb", [V, E], F32, kind="ExternalInput")
    h_t = nc.dram_tensor("h0", [BL, H], F32, kind="ExternalInput")
    encT_t = nc.dram_tensor("encT", [H, BL, S], FP16, kind="ExternalInput")
    wihT_t = nc.dram_tensor("wihT", [E, G], F32R, kind="ExternalInput")
    whhT_t = nc.dram_tensor("whhT", [HT, P, G], F32R, kind="ExternalInput")
    bih_t = nc.dram_tensor("bih", [1, G], F32, kind="ExternalInput")
    bhh_t = nc.dram_tensor("bhh", [1, G], F32, kind="ExternalInput")
    wh2_t = nc.dram_tensor("wh2", [P, HT], F32, kind="ExternalInput")
    ws2_t = nc.dram_tensor("ws2", [P, HT], F32, kind="ExternalInput")
    av2_t = nc.dram_tensor("av2", [P, HT], F32, kind="ExternalInput")
    attb_t = nc.dram_tensor("attb", [1, 1], F32, kind="ExternalInput")
    genw_t = nc.dram_tensor("genw", [P, 5], F32R, kind="ExternalInput")
    genb_t = nc.dram_tensor("genb", [1, 1], F32, kind="ExternalInput")
    outhWT_t = nc.dram_tensor("outhWT", [4, P, H], F32R, kind="ExternalInput")
    outhb_t = nc.dram_tensor("outhb", [1, H], F32, kind="ExternalInput")
    wvT_t = nc.dram_tensor("wvT", [H, VL], FP16, kind="ExternalInput")
    bv_t = nc.dram_tensor("bv", [1, VL], FP16, kind="ExternalInput")

    hn_o = nc.dram_tensor("hn_o", [BL, H], F32, kind="ExternalOutput")
    pg_o = nc.dram_tensor("pg_o", [BL, 1], F32, kind="ExternalOutput")
    ad_o = nc.dram_tensor("ad_o", [BL, S], F32, kind="ExternalOutput")
    pv_o = nc.dram_tensor("pv_o", [B, VL], F32, kind="ExternalOutput")

    with tile.TileContext(nc) as tc, ExitStack() as ctx:
        const = ctx.enter_context(tc.tile_pool(name="const", bufs=1))
        tp_ps = ctx.enter_context(tc.tile_pool(name="tp_ps", bufs=1, space="PSUM"))
        mm_ps = ctx.enter_context(tc.tile_pool(name="mm_ps", bufs=6, space="PSUM"))
        dram = ctx.enter_context(tc.tile_pool(name="dram", bufs=1, space="DRAM"))

        ident = const.tile([P, P], F32)
        make_identity(nc, ident[:])

        def transpose128(dst_ap, src_ap):
            """dst[j,i] = src[i,j] for a [128,128] block (via PE + DVE copy)."""
            ps = tp_ps.tile([P, P], F32, tag="tp", name="tp")
            nc.tensor.transpose(ps[:], src_ap, ident[:])
            nc.vector.tensor_copy(dst_ap, ps[:])

        # DRAM bounce buffers
        sc_dram = dram.tile([1, BL * S], F32, name="sc_dram")
        ad_dram = dram.tile([BL, S], FP16, name="ad_dram")
        ag_in = dram.tile([HT * 2 * P, P], FP16, name="ag_in")
        ag_out = dram.tile([NC * HT * 2 * P, P], FP16, name="ag_out")
        den_in = [dram.tile([P, 1], F32, name=f"den_in{r}") for r in range(NC)]
        den_out = [dram.tile([P, 1], F32, name=f"den_out{r}") for r in range(NC)]

        # tiles that must survive into the vocab phase
        decTall = const.tile([P, HT, 2, NC * P], FP16)
        ones_r = const.tile([1, P], F32R)

        with tc.tile_pool(name="w", bufs=1) as sb:
            # ---------------- prologue loads ----------------
            tok_sb = sb.tile([P, 1], I32)
            nc.sync.dma_start(tok_sb[:], tok.ap())
            x_sb = sb.tile([P, E], F32)
            nc.gpsimd.indirect_dma_start(
                out=x_sb[:], out_offset=None, in_=emb_t.ap(),
                in_offset=bass.IndirectOffsetOnAxis(ap=tok_sb[:, :1], axis=0))

            h_sb = sb.tile([P, H], F32)
            nc.sync.dma_start(h_sb[:], h_t.ap())
            wih_sb = sb.tile([P, G], F32R)
            nc.sync.dma_start(wih_sb[:], wihT_t.ap())
            whh_sb = sb.tile([P, HT, G], F32R)
            nc.sync.dma_start(whh_sb[:], whhT_t.ap().rearrange("t p n -> p t n"))
            wh2_sb = sb.tile([P, HT], F32)
            nc.sync.dma_start(wh2_sb[:], wh2_t.ap())
            ws2_sb = sb.tile([P, HT], F32)
            nc.sync.dma_start(ws2_sb[:], ws2_t.ap())
            av2_sb = sb.tile([P, HT], F32)
            nc.sync.dma_start(av2_sb[:], av2_t.ap())
            genw_sb = sb.tile([P, 5], F32R)
            nc.sync.dma_start(genw_sb[:], genw_t.ap())
            outhW_sb = sb.tile([P, 4, H], F32R)
            nc.sync.dma_start(outhW_sb[:], outhWT_t.ap().rearrange("t p n -> p t n"))

            NR = 2 * G + H + 2
            gru_ctx = ExitStack()
            grup = gru_ctx.enter_context(tc.tile_pool(name="grup", bufs=1))
            rows_sb = grup.tile([1, NR], F32)  # bih | bhh | outhb | attb | genb
            nc.sync.dma_start(rows_sb[:1, 0:G], bih_t.ap())
            nc.sync.dma_start(rows_sb[:1, G:2 * G], bhh_t.ap())
            nc.sync.dma_start(rows_sb[:1, 2 * G:2 * G + H], outhb_t.ap())
            nc.sync.dma_start(rows_sb[:1, 2 * G + H:2 * G + H + 1], attb_t.ap())
            nc.sync.dma_start(rows_sb[:1, 2 * G + H + 1:NR], genb_t.ap())

            ones_f = grup.tile([1, P], F32)
            nc.vector.memset(ones_f[:], 1.0)
            nc.vector.tensor_copy(ones_r[:], ones_f[:])

            # replicated bias tiles
            brz_row = grup.tile([1, 2 * H], F32)   # (b_ih + b_hh)[0:512]
            nc.vector.tensor_add(brz_row[:1, :], rows_sb[:1, 0:2 * H],
                                 rows_sb[:1, G:G + 2 * H])
            brz_rep = grup.tile([P, 2 * H], F32)
            nc.gpsimd.partition_broadcast(brz_rep[:], brz_row[:1, :])
            bihn_rep = grup.tile([P, H], F32)
            nc.gpsimd.partition_broadcast(bihn_rep[:], rows_sb[:1, 2 * H:G])
            bhhn_rep = grup.tile([P, H], F32)
            nc.gpsimd.partition_broadcast(bhhn_rep[:], rows_sb[:1, G + 2 * H:2 * G])
            outhb_rep = sb.tile([P, H], F32)
            nc.gpsimd.partition_broadcast(outhb_rep[:], rows_sb[:1, 2 * G:2 * G + H])
            attb_pp = sb.tile([P, 1], F32)
            nc.gpsimd.partition_broadcast(attb_pp[:],
                                          rows_sb[:1, 2 * G + H:2 * G + H + 1])
            genb_pp = sb.tile([P, 1], F32)
            nc.gpsimd.partition_broadcast(genb_pp[:], rows_sb[:1, 2 * G + H + 1:NR])

            # attn_v replicated into lhsT form [P(h), HT, 128(m)]
            av_rep = sb.tile([P, HT, P], FP16)
            for t in range(HT):
                nc.vector.tensor_copy(av_rep[:, t, :],
                                      av2_sb[:, t:t + 1].broadcast_to([P, P]))

            # ---------------- GRU ----------------
            xT_sb = sb.tile([P, E], F32R)
            transpose128(xT_sb[:], x_sb[:])
            hT_sb = sb.tile([P, HT, P], F32R)
            for t in range(HT):
                transpose128(hT_sb[:, t, :], h_sb[:, t * P:(t + 1) * P])

            hn_sb = grup.tile([P, H], F32)
            # gate chunks: a = cols [0:512] (r,z), b = cols [512:768] (n)
            gi_a = mm_ps.tile([P, VCH], F32, tag="mm", name="gi_a")
            gi_b = mm_ps.tile([P, VCH], F32, tag="mm", name="gi_b")
            gh_a = mm_ps.tile([P, VCH], F32, tag="mm", name="gh_a")
            gh_b = mm_ps.tile([P, VCH], F32, tag="mm", name="gh_b")
            for (pi, ph, n0, n1) in [(gi_a, gh_a, 0, 512), (gi_b, gh_b, 512, G)]:
                w = n1 - n0
                nc.tensor.matmul(pi[:, :w], xT_sb[:], wih_sb[:, n0:n1],
                                 start=True, stop=True)
                for t in range(HT):
                    nc.tensor.matmul(ph[:, :w], hT_sb[:, t, :],
                                     whh_sb[:, t, n0:n1],
                                     start=(t == 0), stop=(t == HT - 1))

            # r,z = sigmoid(gi + gh + bih + bhh) over [0:512]
            rz_sb = grup.tile([P, 2 * H], F32)
            nc.scalar.copy(rz_sb[:], gi_a[:, :2 * H])
            nc.vector.tensor_add(rz_sb[:], rz_sb[:], gh_a[:, :2 * H])
            nc.vector.tensor_add(rz_sb[:], rz_sb[:], brz_rep[:])
            # sigmoid(x) = 0.5*tanh(0.5*x) + 0.5 (keeps ACT on the Tanh table)
            nc.scalar.activation(rz_sb[:], rz_sb[:], ACTF.Tanh, scale=0.5)
            nc.vector.tensor_scalar(rz_sb[:], rz_sb[:], 0.5, 0.5,
                                    op0=ALU.mult, op1=ALU.add)
            # n = tanh(i_n + b_ihn + r*(h_n + b_hhn))
            n_sb = grup.tile([P, H], F32)
            nc.vector.tensor_add(n_sb[:], gh_b[:, :H], bhhn_rep[:])
            nc.vector.tensor_mul(n_sb[:], n_sb[:], rz_sb[:, 0:H])
            nc.vector.tensor_add(n_sb[:], n_sb[:], gi_b[:, :H])
            nc.vector.tensor_add(n_sb[:], n_sb[:], bihn_rep[:])
            nc.scalar.activation(n_sb[:], n_sb[:], ACTF.Tanh)
            # h_new = n + z*(h - n)
            nc.vector.tensor_sub(hn_sb[:], h_sb[:], n_sb[:])
            nc.vector.tensor_mul(hn_sb[:], hn_sb[:], rz_sb[:, H:2 * H])
            nc.vector.tensor_add(hn_sb[:], hn_sb[:], n_sb[:])
            nc.scalar.dma_start(hn_o.ap(), hn_sb[:])

            hnT_sb = sb.tile([P, HT, P], F32R)
            for t in range(HT):
                transpose128(hnT_sb[:, t, :], hn_sb[:, t * P:(t + 1) * P])
            # u'_T[h, b] = (w_s/w_h)*h_newT + att_bias/w_h so that
            # tanh(w_h*E + u) == tanh(w_h*(E + u')): w_h moves into ACT scale
            rwh2 = sb.tile([P, HT], F32)
            nc.vector.reciprocal(rwh2[:], wh2_sb[:])
            wsw = sb.tile([P, HT], F32)
            nc.vector.tensor_mul(wsw[:], ws2_sb[:], rwh2[:])
            abw = sb.tile([P, HT], F32)
            nc.vector.tensor_mul(abw[:], attb_pp[:, :1].broadcast_to([P, HT]),
                                 rwh2[:])
            uT_sb = sb.tile([P, HT, P], FP16)
            for t in range(HT):
                nc.vector.tensor_scalar(uT_sb[:, t, :], hnT_sb[:, t, :],
                                        wsw[:, t:t + 1], abw[:, t:t + 1],
                                        op0=ALU.mult, op1=ALU.add)

            gru_ctx.close()

            # ---------------- attention ----------------
            # processed in NBC independent batch chunks of BC rows:
            # scores -> per-chunk softmax -> context, streaming encoder chunks
            ctxT_f = sb.tile([P, HT, P], F32)
            ctxT_sb = sb.tile([P, HT, P], F32R)
            CSZ = BC * S  # 1600

            with tc.tile_pool(name="enc", bufs=8) as encp, \
                 tc.tile_pool(name="attw", bufs=3) as attp:
                esrc = encT_t.ap().rearrange("(t p) b s -> p t (b s)", t=HT)
                enc_tiles = []
                for c in range(NBC):
                    cs = slice(c * CSZ, (c + 1) * CSZ)
                    enc_c = encp.tile([P, HT, CSZ], FP16, tag="encc",
                                      name=f"enc_c{c}")
                    nc.sync.dma_start(enc_c[:], esrc[:, :, cs])
                    enc_tiles.append(enc_c)
                for c in range(NBC):
                    bsl = slice(c * BC, (c + 1) * BC)
                    enc_c = enc_tiles[c]
                    att_c = attp.tile([P, HT, CSZ], FP16, tag="attc", name="att_c")
                    for t in range(HT):
                        # att = tanh(w_h*(E + u'))  (u' broadcast over s;
                        # w_h applied via the per-partition ACT scale)
                        nc.vector.tensor_tensor(
                            att_c[:, t, :].rearrange("p (b s) -> p b s", b=BC),
                            enc_c[:, t, :].rearrange("p (b s) -> p b s", b=BC),
                            uT_sb[:, t, bsl].unsqueeze(2)
                                .broadcast_to([P, BC, S]),
                            op=ALU.add)
                        nc.scalar.activation(att_c[:, t, :], att_c[:, t, :],
                                             ACTF.Tanh,
                                             scale=wh2_sb[:, t:t + 1])
                    # scores chunk: av . att -> rows replicated; keep row 0
                    for k, n0 in enumerate(range(0, CSZ, SCN)):
                        sc_ps = mm_ps.tile([P, VCH], F32, tag="mm", name="sc_ps")
                        for t in range(HT):
                            nc.tensor.matmul(sc_ps[:, :SCN], av_rep[:, t, :],
                                             att_c[:, t, n0:n0 + SCN],
                                             start=(t == 0), stop=(t == HT - 1))
                        sc_row = attp.tile([1, SCN], F32, tag="scrow", name="sc_row")
                        if k % 2:
                            nc.vector.tensor_copy(sc_row[:1, :], sc_ps[:1, :SCN])
                        else:
                            nc.scalar.copy(sc_row[:1, :], sc_ps[:1, :SCN])
                        off = c * CSZ + n0
                        nc.scalar.dma_start(sc_dram[:1, off:off + SCN],
                                            sc_row[:1, :])

                    # scatter scores [1, CSZ] -> [BC, S] and per-chunk softmax
                    scores_c = attp.tile([BC, S], F32, tag="scc", name="scores_c")
                    nc.scalar.dma_start(
                        scores_c[:],
                        sc_dram[:1, c * CSZ:(c + 1) * CSZ]
                            .rearrange("o (b s) -> (o b) s", b=BC))
                    negmx = attp.tile([BC, 1], F32, tag="ngm", name="negmx")
                    nc.vector.tensor_reduce(negmx[:], scores_c[:], axis=AX.X,
                                            op=ALU.max, negate=True)
                    ssum = attp.tile([BC, 1], F32, tag="ssum", name="ssum")
                    ad_c = attp.tile([BC, S], F32, tag="adc", name="ad_c")
                    nc.scalar.activation(ad_c[:], scores_c[:], ACTF.Exp,
                                         bias=negmx[:, :1], accum_out=ssum[:, :1])
                    srec = attp.tile([BC, 1], F32, tag="srec", name="srec")
                    nc.vector.reciprocal(srec[:], ssum[:])
                    nc.vector.tensor_scalar(ad_c[:], ad_c[:], srec[:, :1], None,
                                            op0=ALU.mult)
                    nc.scalar.dma_start(ad_o.ap()[bsl, :], ad_c[:])
                    ad16_c = attp.tile([BC, S], FP16, tag="ad16", name="ad16_c")
                    nc.vector.tensor_copy(ad16_c[:], ad_c[:])
                    nc.scalar.dma_start(ad_dram[bsl, :], ad16_c[:])

                    # context: ctxT[h, b] = sum_s att_dist[b, s] * encT[h, b, s]
                    adf = attp.tile([1, CSZ], FP16, tag="adf", name="adf")
                    nc.scalar.dma_start(
                        adf[:1, :],
                        ad_dram[bsl, :].rearrange("b s -> (b s)").unsqueeze(0))
                    arep = attp.tile([P, CSZ], FP16, tag="arep", name="arep")
                    nc.gpsimd.partition_broadcast(arep[:], adf[:1, :])
                    for t in range(HT):
                        prod = attp.tile([P, CSZ], FP16, tag="prod", name="prod")
                        nc.vector.tensor_mul(prod[:], enc_c[:, t, :], arep[:])
                        nc.vector.tensor_reduce(
                            ctxT_f[:, t, bsl],
                            prod[:].rearrange("p (b s) -> p b s", b=BC),
                            axis=AX.X, op=ALU.add)

            nc.vector.tensor_copy(ctxT_sb[:], ctxT_f[:])

            # ---------------- dec_out, p_gen ----------------
            lhs_list = [hnT_sb[:, 0, :], hnT_sb[:, 1, :],
                        ctxT_sb[:, 0, :], ctxT_sb[:, 1, :]]
            od_ps = mm_ps.tile([P, VCH], F32, tag="mm", name="od_ps")
            for kt in range(4):
                nc.tensor.matmul(od_ps[:, :H], lhs_list[kt], outhW_sb[:, kt, :],
                                 start=(kt == 0), stop=(kt == 3))
            dec_sb = sb.tile([P, H], F32)
            nc.vector.tensor_add(dec_sb[:], od_ps[:, :H], outhb_rep[:])

            pg_ps = mm_ps.tile([P, VCH], F32, tag="mm", name="pg_ps")
            for i, lhs in enumerate(lhs_list + [xT_sb[:]]):
                nc.tensor.matmul(pg_ps[:, :1], lhs.bitcast(F32),
                                 genw_sb[:, i:i + 1].bitcast(F32),
                                 start=(i == 0), stop=(i == 4))
            # sigmoid(x) = 1 / (1 + exp(-x)); keeps ACT on the Exp table
            pg_sb = sb.tile([P, 1], F32)
            neggenb = sb.tile([P, 1], F32)
            nc.vector.tensor_scalar(neggenb[:], genb_pp[:], -1.0, None,
                                    op0=ALU.mult)
            nc.scalar.activation(pg_sb[:], pg_ps[:, :1], ACTF.Exp,
                                 scale=-1.0, bias=neggenb[:, :1])
            nc.vector.tensor_scalar(pg_sb[:], pg_sb[:], 1.0, None, op0=ALU.add)
            nc.vector.reciprocal(pg_sb[:], pg_sb[:])
            nc.scalar.dma_start(pg_o.ap(), pg_sb[:])

            # decT (fp16 + fp16 residual) -> allgather
            decT_sb = sb.tile([P, HT, 2, P], FP16)
            for t in range(HT):
                psT = tp_ps.tile([P, P], F32, tag="tp", name="tp")
                nc.tensor.transpose(psT[:], dec_sb[:, t * P:(t + 1) * P],
                                    ident[:])
                nc.vector.tensor_copy(decT_sb[:, t, 0, :], psT[:])
                res_f = sb.tile([P, P], F32, tag="resf", name="res_f")
                nc.vector.tensor_sub(res_f[:], psT[:], decT_sb[:, t, 0, :])
                nc.vector.tensor_copy(decT_sb[:, t, 1, :], res_f[:])
            nc.sync.dma_start(ag_in[:].rearrange("(t u p) b -> p t u b",
                                                 t=HT, u=2),
                              decT_sb[:])

            # keep the PE HAM-warm through the AllGather wait: a chain of
            # bf16 matmuls gated on decT (they run during the collective)
            warm_lhs = const.tile([P, P], BF16)
            nc.vector.tensor_copy(warm_lhs[:], decT_sb[:, 0, :].bitcast(F32))
            warm_rhs = const.tile([P, VCH], BF16)
            nc.vector.tensor_copy(warm_rhs[:],
                                  warm_lhs[:, :1].broadcast_to([P, VCH]))
            warm_ps = tp_ps.tile([P, VCH], F32, tag="warm", name="warm_ps",
                                 bufs=1)
            for i in range(250):
                nc.tensor.matmul(warm_ps[:], warm_lhs[:], warm_rhs[:],
                                 start=True, stop=True)
            warm_out = const.tile([P, 32], F32)
            nc.vector.tensor_copy(warm_out[:], warm_ps[:, :32])
        # `w` pool (and enc) closed: SBUF free for vocab phase

        nc.gpsimd.collective_compute(
            "AllGather", ALU.bypass,
            replica_groups=[list(range(NC))],
            ins=[ag_in[:].opt()], outs=[ag_out[:].opt()])
        ag_view = ag_out[:].rearrange("(c t u p) b -> p t u c b", c=NC, t=HT,
                                      u=2)
        for t in range(HT):
            for u in range(2):
                nc.scalar.dma_start(
                    decTall[:, t, u, :].rearrange("p (c b) -> p c b", c=NC),
                    ag_view[:, t, u, :, :])

        # ---------------- vocab matmul + softmax (vocab-sharded) ----------------
        GROUPS = [[0, 1, 2, 3, 4], [5, 6, 7]]
        with tc.tile_pool(name="vocab", bufs=1) as vb, \
             tc.tile_pool(name="expp", bufs=1) as expp, \
             tc.tile_pool(name="vsmall", bufs=3) as vsm:
            wv_sb = vb.tile([P, HT, VL], FP16)
            wsrc = wvT_t.ap().rearrange("(t p) v -> p t v", t=HT)
            for c in range(4):
                cs = slice(c * VL // 4, (c + 1) * VL // 4)
                nc.sync.dma_start(wv_sb[:, :, cs], wsrc[:, :, cs])
            bv_sb = vb.tile([1, VL], FP16)
            nc.sync.dma_start(bv_sb[:1, :], bv_t.ap())
            bvrep = vb.tile([P, VL], FP16)
            nc.gpsimd.partition_broadcast(bvrep[:], bv_sb[:1, :])
            inv128 = vb.tile([P, P], FP16)
            nc.vector.memset(inv128[:], 1.0 / P)

            # bridge dummies: keep PE warm across the decTall load
            for i in range(15):
                wp2 = tp_ps.tile([P, VCH], F32, tag="warm", name="warm2",
                                 bufs=1)
                nc.tensor.matmul(wp2[:], decTall[:, 0, 0, :P],
                                 wv_sb[:, 0, :VCH], start=True, stop=True)

            dens = vb.tile([P, NC], F32)
            recs = vb.tile([P, NC], F32)
            expbs = [expp.tile([P, VL], BF16, name=f"expb{r}") for r in range(NC)]
            gden_in = [dram.tile([P, len(g)], F32, name=f"gden_in{gi}")
                       for gi, g in enumerate(GROUPS)]
            gden_out = [dram.tile([P, len(g)], F32, name=f"gden_out{gi}")
                        for gi, g in enumerate(GROUPS)]

            def compute_r(r):
                expb = expbs[r]
                parts = vsm.tile([P, NVCH], F32, tag="parts", name="parts")
                lhs = [decTall[:, t, u, r * P:(r + 1) * P]
                       for t in range(HT) for u in range(2)]
                for ch in range(NVCH):
                    c0 = ch * VCH
                    cw = min(VCH, VL - c0)
                    ps = mm_ps.tile([P, VCH], F32, tag="mm", name="vps")
                    nc.tensor.matmul(ps[:, :cw], lhs[0], wv_sb[:, 0, c0:c0 + cw],
                                     start=True, stop=False)
                    nc.tensor.matmul(ps[:, :cw], lhs[2], wv_sb[:, 1, c0:c0 + cw],
                                     start=False, stop=False)
                    nc.tensor.matmul(ps[:, :cw], lhs[1], wv_sb[:, 0, c0:c0 + cw],
                                     start=False, stop=False)
                    nc.tensor.matmul(ps[:, :cw], lhs[3], wv_sb[:, 1, c0:c0 + cw],
                                     start=False, stop=False)
                    nc.tensor.matmul(ps[:, :cw], inv128[:],
                                     bvrep[:, c0:c0 + cw],
                                     start=False, stop=True)
                    nc.scalar.activation(expb[:, c0:c0 + cw], ps[:, :cw],
                                         ACTF.Exp,
                                         accum_out=parts[:, ch:ch + 1])
                nc.vector.tensor_reduce(dens[:, r:r + 1], parts[:], axis=AX.X,
                                        op=ALU.add)

            def norm_group(gi):
                g = GROUPS[gi]
                dsum = vsm.tile([P, len(g)], F32, tag="dsum", name="dsum")
                nc.scalar.dma_start(dsum[:, :len(g)], gden_out[gi][:])
                nc.vector.reciprocal(recs[:, g[0]:g[-1] + 1], dsum[:, :len(g)])
                for r in g:
                    for ch in range(NVCH):
                        c0 = ch * VCH
                        cw = min(VCH, VL - c0)
                        nrm = vsm.tile([P, VCH], F32, tag="nrm", name="nrm",
                                       bufs=4)
                        nc.vector.tensor_scalar(nrm[:, :cw],
                                                expbs[r][:, c0:c0 + cw],
                                                recs[:, r:r + 1], None,
                                                op0=ALU.mult)
                        eng = nc.scalar if ch % 2 == 0 else nc.sync
                        eng.dma_start(
                            pv_o.ap()[r * P:(r + 1) * P, c0:c0 + cw],
                            nrm[:, :cw])

            for gi, g in enumerate(GROUPS):
                for j, r in enumerate(g):
                    compute_r(r)
                    # overlap the previous group's normalize + output DMA
                    # with this group's compute (after the AR has landed)
                    if gi > 0 and j == 1:
                        norm_group(gi - 1)
                nc.sync.dma_start(gden_in[gi][:], dens[:, g[0]:g[-1] + 1])
                nc.gpsimd.collective_compute(
                    "AllReduce", ALU.add,
                    replica_groups=[list(range(NC))],
                    ins=[gden_in[gi][:].opt()], outs=[gden_out[gi][:].opt()])
            norm_group(len(GROUPS) - 1)

    nc.compile()
    return nc


def _prep_inputs(input_token, last_decoder_hidden, encoder_states, emb,
                 W_ih, W_hh, b_ih, b_hh, w_h, w_s, att_bias, attn_v,
                 gen_W, gen_b, outh_W, outh_b, outv_W, outv_b):
    f = np.float32
    emb = np.ascontiguousarray(emb, dtype=f)
    wihT = np.ascontiguousarray(np.asarray(W_ih, f).T)               # [128, 768]
    whhT = np.ascontiguousarray(np.asarray(W_hh, f).T).reshape(HT, P, G)
    outhWT = np.ascontiguousarray(np.asarray(outh_W, f).T).reshape(4, P, H)
    wh2 = np.ascontiguousarray(np.asarray(w_h, f).reshape(HT, P).T)  # [128, 2]
    ws2 = np.ascontiguousarray(np.asarray(w_s, f).reshape(HT, P).T)
    av2 = np.ascontiguousarray(np.asarray(attn_v, f).reshape(HT, P).T)
    genw = np.ascontiguousarray(np.asarray(gen_W, f).reshape(5, P).T)  # [128, 5]
    bih = np.asarray(b_ih, f).reshape(1, G)
    bhh = np.asarray(b_hh, f).reshape(1, G)
    outhb = np.asarray(outh_b, f).reshape(1, H)
    attb = np.asarray(att_bias, f).reshape(1, 1)
    genb = np.asarray(gen_b, f).reshape(1, 1)
    outvT = np.ascontiguousarray(np.asarray(outv_W, f).T).astype(np.float16)
    outvb = np.asarray(outv_b, f).reshape(1, V).astype(np.float16)
    tok_all = np.asarray(input_token).astype(np.int32).reshape(B, 1)
    h_all = np.asarray(last_decoder_hidden, f)
    enc_all = np.asarray(encoder_states, f)

    in_maps = []
    for c in range(NC):
        bs = slice(c * BL, (c + 1) * BL)
        vs = slice(c * VL, (c + 1) * VL)
        encT = np.ascontiguousarray(
            enc_all[bs].transpose(2, 0, 1)).astype(np.float16)  # [H, BL, S]
        in_maps.append({
            "tok": tok_all[bs], "emb": emb, "h0": np.ascontiguousarray(h_all[bs]),
            "encT": encT, "wihT": wihT, "whhT": whhT, "bih": bih, "bhh": bhh,
            "wh2": wh2, "ws2": ws2, "av2": av2, "attb": attb,
            "genw": genw, "genb": genb, "outhWT": outhWT, "outhb": outhb,
            "wvT": np.ascontiguousarray(outvT[:, vs]),
            "bv": np.ascontiguousarray(outvb[:, vs]),
        })
    return in_maps


def _assemble(results):
    hn = np.concatenate([r["hn_o"] for r in results], axis=0)        # [B, H]
    pg = np.concatenate([r["pg_o"] for r in results], axis=0)        # [B, 1]
    ad = np.concatenate([r["ad_o"] for r in results], axis=0)        # [B, S]
    pv = np.concatenate([r["pv_o"] for r in results], axis=1)        # [B, V]
    return hn[None], pg, pv, ad


def _run(in_maps, trace=False, tmpdir=None):
    if "nc" not in _CACHE:
        _CACHE["nc"] = build_bass()
    kw = {}
    if trace:
        kw = {"trace": True, "tmpdir": tmpdir}
    res = bass_utils.run_bass_kernel_spmd(
        _CACHE["nc"], in_maps, core_ids=list(range(NC)), **kw)
    return res


def kernel(**inputs):
    in_maps = _prep_inputs(**inputs)
    res = _run(in_maps)
    return _assemble(res.results)


def kernel_traced(tmpdir, **inputs):
    """Like kernel() but returns (outputs, BassKernelResults) with NTFF profile."""
    in_maps = _prep_inputs(**inputs)
    res = _run(in_maps, trace=True, tmpdir=tmpdir)
    return _assemble(res.results), res
